# revision 3
# baseline (speedup 1.0000x reference)
"""Differentiable random-forest layer (inference path) on 8 Trainium2 cores.

Computation (per reference):
    d     = sigmoid(einsum('bf,tfn->btn', x, W))        # [B, T, 255]
    route = prod_l where(IS_LEFT, d[..n..], 1-d[..n..]) # [B, T, 256]
    out   = clip(einsum('btl,tlc->bc', route, P) / T, 0, 1)

Shapes: B=4096, F=1024, T=10 trees, 255 nodes / 256 leaves, C=1000.

Sharding: data-parallel over batch. Each of the 8 cores handles 512 rows;
no collectives are needed (weights/probs are broadcast to every core).

Per-core pipeline (all matmuls bf16 inputs with fp32 PSUM accumulation):
  mm1   : d_logits[b,510] += xT[k,b].T @ W[k, tree-pair]   (PE)
  sig   : d = sigmoid(logits), dbar = sigmoid(-logits)     (ACT, psum->sbuf bf16)
  route : hierarchical doubling, R_{l+1}[2k+s] = R_l[k]*{d,dbar}[node]  (DVE)
  transp: route [b,leaf] -> routeT [leaf,b] via PE transpose + copy
  mm2   : out[b,c] += routeT.T @ P[leaf-chunk]             (PE, over trees)
  store : out = relu(0.1 * psum)  -> DRAM                  (ACT + DMA)
"""

from contextlib import ExitStack

import numpy as np
import ml_dtypes

import concourse.bass as bass
import concourse.bacc as bacc
import concourse.mybir as mybir
import concourse.tile as tile
from concourse.bass_utils import run_bass_kernel_spmd
from concourse.masks import make_identity

N_CORES = 8
B, F, T, NODES, LEAFS, C = 4096, 1024, 10, 255, 256, 1000
B_LOC = B // N_CORES            # 512 batch rows per core
BCH = B_LOC // 128              # 4 batch chunks of 128
KF = F // 128                   # 8 contraction chunks for mm1
TP = T // 2                     # 5 tree-pairs (2 trees -> 510 psum cols)
N_LAYERS = 8

BF16 = mybir.dt.bfloat16
F32 = mybir.dt.float32
Sigmoid = mybir.ActivationFunctionType.Sigmoid
Relu = mybir.ActivationFunctionType.Relu


def build_program() -> bass.Bass:
    nc = bacc.Bacc()

    xT = nc.dram_tensor("xT", [KF, 128, B_LOC], BF16, kind="ExternalInput")
    w = nc.dram_tensor("w", [KF, 128, T * NODES], BF16, kind="ExternalInput")
    p = nc.dram_tensor("p", [2, 128, T * C], BF16, kind="ExternalInput")
    out = nc.dram_tensor("out", [B_LOC, C], F32, kind="ExternalOutput")

    with tile.TileContext(nc) as tc, ExitStack() as ctx:
        consts = ctx.enter_context(tc.tile_pool(name="consts", bufs=1))
        ident = consts.tile([128, 128], BF16)
        make_identity(nc, ident)

        resident = ctx.enter_context(tc.tile_pool(name="resident", bufs=1))
        xT_sb = []
        w_sb = []
        for k in range(KF):
            xk = resident.tile([128, B_LOC], BF16, tag=f"x{k}", name=f"x{k}")
            nc.sync.dma_start(xk[:, :], xT[k])
            xT_sb.append(xk)
            wk = resident.tile([128, T * NODES], BF16, tag=f"w{k}", name=f"w{k}")
            nc.sync.dma_start(wk[:, :], w[k])
            w_sb.append(wk)
        p_sb = []
        for kc in range(2):
            pk = resident.tile([128, T * C], BF16, tag=f"p{kc}", name=f"p{kc}")
            nc.sync.dma_start(pk[:, :], p[kc])
            p_sb.append(pk)

        dpool = ctx.enter_context(tc.tile_pool(name="dps", bufs=1, space="PSUM"))
        tpool = ctx.enter_context(tc.tile_pool(name="tps", bufs=2, space="PSUM"))
        opool = ctx.enter_context(tc.tile_pool(name="ops", bufs=1, space="PSUM"))
        work = ctx.enter_context(tc.tile_pool(name="work", bufs=2))

        for bi in range(BCH):
            bsl = bass.ts(bi, 128)

            # ---- mm1: d logits for all 10 trees, this batch chunk ----
            dps = [
                dpool.tile([128, 2, NODES], F32, tag=f"dps{j}", name=f"dps{j}")
                for j in range(TP)
            ]
            for k in range(KF):
                lhs = xT_sb[k][:, bsl]
                for j in range(TP):
                    nc.tensor.matmul(
                        dps[j][:, :, :],
                        lhs,
                        w_sb[k][:, j * 2 * NODES : (j + 1) * 2 * NODES],
                        start=(k == 0),
                        stop=(k == KF - 1),
                    )

            # ---- sigmoid: d and (1-d) = sigmoid(-x), psum -> sbuf bf16 ----
            d_sb = work.tile([128, T, NODES], BF16, tag="d", name="d")
            db_sb = work.tile([128, T, NODES], BF16, tag="db", name="db")
            for j in range(TP):
                nc.scalar.activation(d_sb[:, 2 * j : 2 * j + 2, :], dps[j][:, :, :], Sigmoid)
                nc.scalar.activation(
                    db_sb[:, 2 * j : 2 * j + 2, :], dps[j][:, :, :], Sigmoid, scale=-1.0
                )

            # ---- routing: hierarchical doubling over 8 layers ----
            Ra = work.tile([128, T, LEAFS], BF16, tag="Ra", name="Ra")
            Rb = work.tile([128, T, LEAFS], BF16, tag="Rb", name="Rb")
            nc.vector.tensor_copy(Ra[:, :, 0:1], d_sb[:, :, 0:1])
            nc.vector.tensor_copy(Ra[:, :, 1:2], db_sb[:, :, 0:1])
            cur, nxt = Ra, Rb
            for l in range(1, N_LAYERS):
                w_l = 1 << l          # prefixes at layer l
                off = w_l - 1         # first node index of layer l
                nc.vector.tensor_mul(
                    nxt[:, :, 0 : 2 * w_l : 2], cur[:, :, 0:w_l], d_sb[:, :, off : off + w_l]
                )
                nc.vector.tensor_mul(
                    nxt[:, :, 1 : 2 * w_l : 2], cur[:, :, 0:w_l], db_sb[:, :, off : off + w_l]
                )
                cur, nxt = nxt, cur
            route = cur  # [128, T, 256] bf16

            # ---- transpose route -> routeT [leaf, b] per (tree, leaf-chunk) ----
            rT = work.tile([128, 2 * T, 128], BF16, tag="rT", name="rT")
            for g in range(3):
                n_in_g = 8 if g < 2 else 2 * T - 16
                tps = tpool.tile([128, 8, 128], BF16, tag="tps", name="tps")
                for jj in range(n_in_g):
                    ch = g * 8 + jj
                    t_, kc = divmod(ch, 2)
                    nc.tensor.transpose(
                        tps[:, jj, :], route[:, t_, kc * 128 : (kc + 1) * 128], ident
                    )
                nc.vector.tensor_copy(
                    rT[:, g * 8 : g * 8 + n_in_g, :], tps[:, 0:n_in_g, :]
                )

            # ---- mm2: out[b, c] += routeT.T @ P, accumulated over trees ----
            osb = work.tile([128, C], F32, tag="osb", name="osb")
            for n0, nsz in ((0, 512), (512, C - 512)):
                ops = opool.tile([128, 512], F32, tag="ops", name="ops")
                for t_ in range(T):
                    for kc in range(2):
                        nc.tensor.matmul(
                            ops[:, 0:nsz],
                            rT[:, 2 * t_ + kc, :],
                            p_sb[kc][:, t_ * C + n0 : t_ * C + n0 + nsz],
                            start=(t_ == 0 and kc == 0),
                            stop=(t_ == T - 1 and kc == 1),
                        )
                # mean over trees (x0.1) + clip lower bound; upper bound is
                # provably inactive (outputs <= max(P) ~ 2e-3)
                nc.scalar.activation(osb[:, n0 : n0 + nsz], ops[:, 0:nsz], Relu, scale=1.0 / T)

            nc.sync.dma_start(out[bsl, :], osb[:, :])

    nc.finalize()
    return nc


_CACHED_NC = None


def _get_nc() -> bass.Bass:
    global _CACHED_NC
    if _CACHED_NC is None:
        _CACHED_NC = build_program()
    return _CACHED_NC


def _prep_inputs(l_input, cnn_w, final_probabilities):
    bf = ml_dtypes.bfloat16
    x = np.ascontiguousarray(np.asarray(l_input, dtype=np.float32))
    W = np.asarray(cnn_w, dtype=np.float32)
    P = np.asarray(final_probabilities, dtype=np.float32)

    # x [B, F] -> xT [KF, 128, B] (transposed, contraction-chunk major)
    xT = np.ascontiguousarray(x.T).astype(bf).reshape(KF, 128, B)
    # W [T, F, N] -> [F, T, N] -> [KF, 128, T*N]
    Wr = np.ascontiguousarray(W.transpose(1, 0, 2)).astype(bf).reshape(KF, 128, T * NODES)
    # P [T, 256, C] -> [leaf-chunk, 128, T*C]
    Pr = np.ascontiguousarray(
        P.reshape(T, 2, 128, C).transpose(1, 2, 0, 3)
    ).astype(bf).reshape(2, 128, T * C)
    return xT, Wr, Pr


def _run(inputs, trace=False, trace_cores=None):
    xT, Wr, Pr = _prep_inputs(
        inputs["l_input"], inputs["cnn_w"], inputs["final_probabilities"]
    )
    in_maps = [
        {
            "xT": np.ascontiguousarray(xT[:, :, c * B_LOC : (c + 1) * B_LOC]),
            "w": Wr,
            "p": Pr,
        }
        for c in range(N_CORES)
    ]
    res = run_bass_kernel_spmd(
        _get_nc(),
        in_maps,
        core_ids=list(range(N_CORES)),
        trace=trace,
        trace_cores=trace_cores,
    )
    out = np.concatenate([res.results[c]["out"] for c in range(N_CORES)], axis=0)
    return out, res


def kernel(**inputs) -> np.ndarray:
    out, _ = _run(inputs)
    return out


# revision 4
# speedup vs baseline: 1.0375x; 1.0375x over previous
"""Differentiable random-forest layer (inference path) on 8 Trainium2 cores.

Computation (per reference):
    d     = sigmoid(einsum('bf,tfn->btn', x, W))        # [B, T, 255]
    route = prod_l where(IS_LEFT, d[..n..], 1-d[..n..]) # [B, T, 256]
    out   = clip(einsum('btl,tlc->bc', route, P) / T, 0, 1)

Shapes: B=4096, F=1024, T=10 trees, 255 nodes / 256 leaves, C=1000.

Sharding: data-parallel over batch. Each of the 8 cores handles 512 rows;
no collectives are needed (weights/probs are broadcast to every core).

Per-core pipeline (all matmuls bf16 inputs with fp32 PSUM accumulation):
  mm1   : d_logits[b,510] += xT[k,b].T @ W[k, tree-pair]   (PE)
  sig   : d = sigmoid(logits), dbar = sigmoid(-logits)     (ACT, psum->sbuf bf16)
  route : hierarchical doubling R_{l+1} = [R_l*d_l, R_l*dbar_l]  (DVE)
  transp: route [b,leaf] -> routeT [leaf,b] via batched XBAR DMA transpose
  mm2   : out[b,c] += routeT.T @ P[leaf-chunk]             (PE, over trees)
  store : out = relu(0.1 * psum)  -> DRAM                  (ACT + DMA)

The routing uses the "concat" (decision-bit-as-LSB) leaf ordering so every
DVE read/write is contiguous; the host pre-permutes W's node axis (per-layer
bit-reversal) and P's leaf axis (8-bit reversal) to compensate, which is free.
"""

from contextlib import ExitStack

import numpy as np
import ml_dtypes

import concourse.bass as bass
import concourse.bacc as bacc
import concourse.mybir as mybir
import concourse.tile as tile
from concourse.bass_utils import run_bass_kernel_spmd

N_CORES = 8
B, F, T, NODES, LEAFS, C = 4096, 1024, 10, 255, 256, 1000
B_LOC = B // N_CORES            # 512 batch rows per core
BCH = B_LOC // 128              # 4 batch chunks of 128
KF = F // 128                   # 8 contraction chunks for mm1
TP = T // 2                     # 5 tree-pairs (2 trees -> 510 psum cols)
N_LAYERS = 8

BF16 = mybir.dt.bfloat16
F32 = mybir.dt.float32
Sigmoid = mybir.ActivationFunctionType.Sigmoid
Relu = mybir.ActivationFunctionType.Relu


def _bitrev(x: int, bits: int) -> int:
    r = 0
    for _ in range(bits):
        r = (r << 1) | (x & 1)
        x >>= 1
    return r


# Node-axis permutation: d'[.., off+q] = d[.., off+bitrev_l(q)] per layer l
NODE_PERM = np.empty(NODES, dtype=np.int64)
for _l in range(N_LAYERS):
    _off = (1 << _l) - 1
    for _q in range(1 << _l):
        NODE_PERM[_off + _q] = _off + _bitrev(_q, _l)
# Leaf-axis permutation: P'[t, q, :] = P[t, bitrev_8(q), :]
LEAF_PERM = np.array([_bitrev(q, N_LAYERS) for q in range(LEAFS)], dtype=np.int64)


def build_program() -> bass.Bass:
    nc = bacc.Bacc()

    xT = nc.dram_tensor("xT", [KF, 128, B_LOC], BF16, kind="ExternalInput")
    w = nc.dram_tensor("w", [KF, 128, T * NODES], BF16, kind="ExternalInput")
    p = nc.dram_tensor("p", [2, 128, T * C], BF16, kind="ExternalInput")
    out = nc.dram_tensor("out", [B_LOC, C], F32, kind="ExternalOutput")

    with tile.TileContext(nc) as tc, ExitStack() as ctx:
        resident = ctx.enter_context(tc.tile_pool(name="resident", bufs=1))
        xT_sb = [
            resident.tile([128, B_LOC], BF16, tag=f"x{k}", name=f"x{k}")
            for k in range(KF)
        ]
        w_sb = [
            resident.tile([128, T * NODES], BF16, tag=f"w{k}", name=f"w{k}")
            for k in range(KF)
        ]
        p_sb = [
            resident.tile([128, T * C], BF16, tag=f"p{kc}", name=f"p{kc}")
            for kc in range(2)
        ]
        # k=0 first: mm1 can start as soon as the first chunks land
        for k in range(KF):
            nc.sync.dma_start(w_sb[k][:, :], w[k])
            nc.sync.dma_start(xT_sb[k][:, :], xT[k])
        for kc in range(2):
            nc.sync.dma_start(p_sb[kc][:, :], p[kc])

        dpool = ctx.enter_context(tc.tile_pool(name="dps", bufs=1, space="PSUM"))
        opool = ctx.enter_context(tc.tile_pool(name="ops", bufs=2, space="PSUM"))
        work = ctx.enter_context(tc.tile_pool(name="work", bufs=2))

        for bi in range(BCH):
            bsl = bass.ts(bi, 128)

            # ---- mm1: d logits for all 10 trees, this batch chunk ----
            dps = [
                dpool.tile([128, 2, NODES], F32, tag=f"dps{j}", name=f"dps{j}")
                for j in range(TP)
            ]
            for k in range(KF):
                lhs = xT_sb[k][:, bsl]
                for j in range(TP):
                    nc.tensor.matmul(
                        dps[j][:, :, :],
                        lhs,
                        w_sb[k][:, j * 2 * NODES : (j + 1) * 2 * NODES],
                        start=(k == 0),
                        stop=(k == KF - 1),
                    )

            # ---- sigmoid: d and (1-d) = sigmoid(-x), psum -> sbuf bf16 ----
            d_sb = work.tile([128, T, NODES], BF16, tag="d", name="d")
            db_sb = work.tile([128, T, NODES], BF16, tag="db", name="db")
            for j in range(TP):
                nc.scalar.activation(d_sb[:, 2 * j : 2 * j + 2, :], dps[j][:, :, :], Sigmoid)
                nc.scalar.activation(
                    db_sb[:, 2 * j : 2 * j + 2, :], dps[j][:, :, :], Sigmoid, scale=-1.0
                )

            # ---- routing: hierarchical doubling, concat ordering ----
            # R_{l+1}[0:w]  = R_l[0:w] * d_l   (decision bit 0 -> left)
            # R_{l+1}[w:2w] = R_l[0:w] * dbar_l
            Ra = work.tile([128, T, LEAFS], BF16, tag="Ra", name="Ra")
            Rb = work.tile([128, T, LEAFS], BF16, tag="Rb", name="Rb")
            routeC = work.tile([128, 2, T, 128], BF16, tag="routeC", name="routeC")
            nc.vector.tensor_copy(Ra[:, :, 0:1], d_sb[:, :, 0:1])
            nc.vector.tensor_copy(Ra[:, :, 1:2], db_sb[:, :, 0:1])
            cur, nxt = Ra, Rb
            for l in range(1, N_LAYERS):
                w_l = 1 << l          # prefixes at layer l
                off = w_l - 1         # first node index of layer l
                if l < N_LAYERS - 1:
                    lo, hi = nxt[:, :, 0:w_l], nxt[:, :, w_l : 2 * w_l]
                else:
                    # last layer: write straight into the transpose-ready
                    # [leaf-chunk, tree, leaf-low] layout
                    lo, hi = routeC[:, 0, :, :], routeC[:, 1, :, :]
                nc.vector.tensor_mul(lo, cur[:, :, 0:w_l], d_sb[:, :, off : off + w_l])
                nc.vector.tensor_mul(hi, cur[:, :, 0:w_l], db_sb[:, :, off : off + w_l])
                cur, nxt = nxt, cur

            # ---- transpose: route [b, leaf] -> routeT [leaf, b], per kc ----
            rT = [
                work.tile([128, T, 128], BF16, tag=f"rT{kc}", name=f"rT{kc}")
                for kc in range(2)
            ]
            for kc in range(2):
                nc.sync.dma_start_transpose(rT[kc][:, :, :], routeC[:, kc])

            # ---- mm2: out[b, c] += routeT.T @ P, accumulated over trees ----
            osb = work.tile([128, C], F32, tag="osb", name="osb")
            for n0, nsz in ((0, 512), (512, C - 512)):
                ops = opool.tile([128, 512], F32, tag="ops", name="ops")
                for t_ in range(T):
                    for kc in range(2):
                        nc.tensor.matmul(
                            ops[:, 0:nsz],
                            rT[kc][:, t_, :],
                            p_sb[kc][:, t_ * C + n0 : t_ * C + n0 + nsz],
                            start=(t_ == 0 and kc == 0),
                            stop=(t_ == T - 1 and kc == 1),
                        )
                # mean over trees (x0.1) + clip lower bound; upper bound is
                # provably inactive (outputs <= max(P) ~ 2e-3)
                nc.scalar.activation(osb[:, n0 : n0 + nsz], ops[:, 0:nsz], Relu, scale=1.0 / T)

            nc.sync.dma_start(out[bsl, :], osb[:, :])

    nc.finalize()
    return nc


_CACHED_NC = None


def _get_nc() -> bass.Bass:
    global _CACHED_NC
    if _CACHED_NC is None:
        _CACHED_NC = build_program()
    return _CACHED_NC


def _prep_inputs(l_input, cnn_w, final_probabilities):
    bf = ml_dtypes.bfloat16
    x = np.ascontiguousarray(np.asarray(l_input, dtype=np.float32))
    W = np.asarray(cnn_w, dtype=np.float32)[:, :, NODE_PERM]
    P = np.asarray(final_probabilities, dtype=np.float32)[:, LEAF_PERM, :]

    # x [B, F] -> xT [KF, 128, B] (transposed, contraction-chunk major)
    xT = np.ascontiguousarray(x.T).astype(bf).reshape(KF, 128, B)
    # W [T, F, N] -> [F, T, N] -> [KF, 128, T*N]
    Wr = np.ascontiguousarray(W.transpose(1, 0, 2)).astype(bf).reshape(KF, 128, T * NODES)
    # P [T, 256, C] -> [leaf-chunk, 128, T*C]
    Pr = np.ascontiguousarray(
        P.reshape(T, 2, 128, C).transpose(1, 2, 0, 3)
    ).astype(bf).reshape(2, 128, T * C)
    return xT, Wr, Pr


def _run(inputs, trace=False, trace_cores=None):
    xT, Wr, Pr = _prep_inputs(
        inputs["l_input"], inputs["cnn_w"], inputs["final_probabilities"]
    )
    in_maps = [
        {
            "xT": np.ascontiguousarray(xT[:, :, c * B_LOC : (c + 1) * B_LOC]),
            "w": Wr,
            "p": Pr,
        }
        for c in range(N_CORES)
    ]
    res = run_bass_kernel_spmd(
        _get_nc(),
        in_maps,
        core_ids=list(range(N_CORES)),
        trace=trace,
        trace_cores=trace_cores,
    )
    out = np.concatenate([res.results[c]["out"] for c in range(N_CORES)], axis=0)
    return out, res


def kernel(**inputs) -> np.ndarray:
    out, _ = _run(inputs)
    return out
